# revision 16
# baseline (speedup 1.0000x reference)
"""BrickedAttention Trainium2 kernel — 8-core SPMD, sequence-parallel.

Sharding: 2 cores per batch element (B=4), each core owns 4096 contiguous
tokens. Pass-2 (shifted windows) needs a 128-token halo on each side, which
the host supplies inside the per-core input (zeros at batch edges, matching
the reference's zero padding exactly). No collectives needed.

Layouts: activations kept feature-major ("xT": [E, tok]) so weight matrices
are the stationary matmul operand and V comes out token-major for free.
All matmul inputs fp16 (full PE rate), fp32 PSUM accumulation.
"""
import numpy as np

import concourse.bacc as bacc
import concourse.bass as bass
import concourse.mybir as mybir
import concourse.tile as tile
from concourse.bass_utils import run_bass_kernel_spmd
from concourse.masks import make_identity

F16 = mybir.dt.float16
F32 = mybir.dt.float32
AF = mybir.ActivationFunctionType
OP = mybir.AluOpType

N_CORES = 8
E = 1024
EC = 8          # E // 128 chunks
W = 256         # window
TCORE = 4096    # tokens per core
TEXT = TCORE + 2 * 128  # with halos
NW1 = TCORE // W        # 16 aligned windows
NW2 = TEXT // W         # 17 shifted windows
EPS = 1e-5
EXP_SHIFT = -8.0        # exp(s + EXP_SHIFT): cancels in softmax, keeps fp16 safe

_cache = {}


def _build(flags):
    use_g1, use_b1, use_g2, use_b2, use_bout = flags
    nc = bacc.Bacc("TRN2", target_bir_lowering=False, debug=False,
                   num_devices=N_CORES)

    def din(name, shape, dt=F32):
        return nc.dram_tensor(name, shape, dt, kind="ExternalInput").ap()

    xt = din("xt", [E, TEXT], F16)          # x^T extended (feature-major)
    xc = din("xc", [TCORE, E], F32)         # center tokens, token-major
    wq0 = din("wq0", [E, E], F16)           # pre-scaled by 1/sqrt(dh)
    wk0 = din("wk0", [E, E], F16)
    wv0 = din("wv0", [E, E], F16)
    wq1 = din("wq1", [E, E], F16)
    wk1 = din("wk1", [E, E], F16)
    wv1 = din("wv1", [E, E], F16)
    wo = din("wo", [E, E], F16)             # pre-scaled by 0.5
    wout = din("wout", [E, E], F16)
    g1v = din("g1v", [E]) if use_g1 else None
    b1v = din("b1v", [E]) if use_b1 else None
    g2v = din("g2v", [E]) if use_g2 else None
    b2v = din("b2v", [E]) if use_b2 else None
    boutv = din("boutv", [E]) if use_bout else None

    out = nc.dram_tensor("out", [TCORE, E], F32, kind="ExternalOutput").ap()
    s1t = nc.dram_tensor("s1t", [E, TCORE], F16).ap()   # attn pass-1 ^T
    s2t = nc.dram_tensor("s2t", [E, TEXT], F16).ap()    # attn pass-2 ^T (ext idx)

    def bcast_row(v):
        # [E] dram vector -> broadcast AP [128, E] (partition step 0)
        return bass.AP(tensor=v.tensor, offset=v.offset, ap=[[0, 128]] + list(v.ap))

    with tile.TileContext(nc) as tc:
        cp = tc.tile_pool(name="const", bufs=1)
        constp = cp.__enter__()
        ones32 = constp.tile([128, 32], F16)
        nc.vector.memset(ones32, 1.0)
        id128 = constp.tile([128, 128], F16)
        make_identity(nc, id128)
        # sel2[p, par, 64g + i] = 1 iff p == 64*par + 32*g: selector that maps
        # 32-replicated per-head denominator rows onto a 64|64 head-pair tile.
        sel2 = constp.tile([128, 2, 128], F16)
        nc.gpsimd.memset(sel2, 0.0)
        nc.gpsimd.affine_select(
            out=sel2.rearrange("p par (g i) -> p par g i", g=2),
            in_=sel2.rearrange("p par (g i) -> p par g i", g=2),
            pattern=[[-64, 2], [-32, 2], [0, 64]],
            compare_op=OP.not_equal,
            fill=1.0,
            base=0,
            channel_multiplier=1)
        eps_t = constp.tile([128, 1], F32)
        nc.vector.memset(eps_t, EPS)
        shift_t = constp.tile([128, 1], F32)
        nc.vector.memset(shift_t, EXP_SHIFT)
        g1b = b1b = g2b = b2b = boutb = None
        if use_g1:
            g1b = constp.tile([128, E], F32)
            nc.sync.dma_start(out=g1b, in_=bcast_row(g1v))
        if use_b1:
            b1b = constp.tile([128, E], F32)
            nc.sync.dma_start(out=b1b, in_=bcast_row(b1v))
        if use_g2:
            g2b = constp.tile([128, E], F32)
            nc.sync.dma_start(out=g2b, in_=bcast_row(g2v))
        if use_b2:
            b2b = constp.tile([128, E], F32)
            nc.sync.dma_start(out=b2b, in_=bcast_row(b2v))
        if use_bout:
            boutb = constp.tile([128, E], F32)
            nc.sync.dma_start(out=boutb, in_=bcast_row(boutv))

        # ---------------- attention passes ----------------
        def attn_pass(p):
            wq_d, wk_d, wv_d = ((wq0, wk0, wv0), (wq1, wk1, wv1))[p]
            nw = (NW1, NW2)[p]
            xoff = (128, 0)[p]
            scr = (s1t, s2t)[p]
            with tc.tile_pool(name=f"w{p}", bufs=1) as wp, \
                 tc.tile_pool(name=f"sb{p}", bufs=2) as sbp, \
                 tc.tile_pool(name=f"pqkv{p}", bufs=2, space="PSUM") as pqkv, \
                 tc.tile_pool(name=f"pss{p}", bufs=2, space="PSUM") as pss, \
                 tc.tile_pool(name=f"pd{p}", bufs=1, space="PSUM") as pd, \
                 tc.tile_pool(name=f"ppv{p}", bufs=1, space="PSUM") as ppv, \
                 tc.tile_pool(name=f"pbc{p}", bufs=1, space="PSUM") as pbc:
                wqs = wp.tile([128, EC, E], F16, name=f"wqs{p}")
                wks = wp.tile([128, EC, E], F16, name=f"wks{p}")
                wvs = wp.tile([128, EC, E], F16, name=f"wvs{p}")
                for dst, src in ((wqs, wq_d), (wks, wk_d), (wvs, wv_d)):
                    nc.sync.dma_start(
                        out=dst, in_=src.rearrange("(c p) n -> p c n", p=128))
                for w in range(nw):
                    base = xoff + W * w
                    X = sbp.tile([128, EC, W], F16, tag="X")
                    nc.sync.dma_start(
                        out=X,
                        in_=xt[:, base:base + W].rearrange(
                            "(c p) t -> p c t", p=128))
                    # q^T, k^T feature-major
                    qT = sbp.tile([128, EC, W], F16, tag="qT")
                    kT = sbp.tile([128, EC, W], F16, tag="kT")
                    for ti, (dst, wsb) in enumerate(((qT, wqs), (kT, wks))):
                        for g in range(4):
                            ps = pqkv.tile([128, 512], F32, tag="qkv")
                            for sub in range(2):
                                m = 2 * g + sub
                                for c in range(EC):
                                    nc.tensor.matmul(
                                        ps[:, sub * W:(sub + 1) * W],
                                        wsb[:, c, m * 128:(m + 1) * 128],
                                        X[:, c, :],
                                        start=(c == 0), stop=(c == EC - 1))
                            eng = nc.vector if (g + 2 * ti) % 2 == 0 else nc.scalar
                            (eng.tensor_copy if eng is nc.vector else eng.copy)(
                                dst[:, 2 * g:2 * g + 2, :].rearrange(
                                    "p a b -> p (a b)"),
                                ps)
                    # v token-major: [tok(128) x kc(2), E]
                    v_sb = sbp.tile([128, 2, E], F16, tag="v")
                    for kc in range(2):
                        for half in range(2):
                            ps = pqkv.tile([128, 512], F32, tag="qkv")
                            for c in range(EC):
                                nc.tensor.matmul(
                                    ps,
                                    X[:, c, kc * 128:(kc + 1) * 128],
                                    wvs[:, c, half * 512:(half + 1) * 512],
                                    start=(c == 0), stop=(c == EC - 1))
                            eng = nc.vector if (kc + half) % 2 == 0 else nc.scalar
                            (eng.tensor_copy if eng is nc.vector else eng.copy)(
                                v_sb[:, kc, half * 512:(half + 1) * 512], ps)
                    # attention, 16 heads
                    d_ps = pd.tile([128, 4, W], F32, tag="d", name=f"d{p}_{w}")
                    pv_sb = sbp.tile([128, 8, W], F16, tag="pv")
                    pvps = None
                    for h in range(16):
                        c = h // 2
                        po = 64 * (h % 2)
                        ss = pss.tile([128, 2 * W], F32, tag="ss")
                        for kc in range(2):
                            nc.tensor.matmul(
                                ss[:, kc * W:(kc + 1) * W],
                                kT[po:po + 64, c, kc * 128:(kc + 1) * 128],
                                qT[po:po + 64, c, :],
                                start=True, stop=True)
                        eS = sbp.tile([128, 2 * W], F16, tag="eS")
                        nc.scalar.activation(out=eS, in_=ss, func=AF.Exp,
                                             bias=shift_t)
                        prow = 32 * (h % 4)
                        dcol = h // 4
                        for kc in range(2):
                            nc.tensor.matmul(
                                d_ps[prow:prow + 32, dcol, :],
                                ones32, eS[:, kc * W:(kc + 1) * W],
                                start=(kc == 0), stop=(kc == 1),
                                tile_position=(0, prow))
                        if h % 2 == 0:
                            pvps = ppv.tile([128, W], F32, tag="pvp",
                                            name=f"pv{p}_{w}_{h}")
                        for kc in range(2):
                            nc.tensor.matmul(
                                pvps[po:po + 64, :],
                                v_sb[:, kc, 64 * h:64 * h + 64],
                                eS[:, kc * W:(kc + 1) * W],
                                start=(kc == 0), stop=(kc == 1))
                        if h % 2 == 1:
                            j = h // 2
                            eng = nc.vector if j % 2 == 0 else nc.scalar
                            (eng.tensor_copy if eng is nc.vector else eng.copy)(
                                pv_sb[:, j, :], pvps)
                    # denominators -> reciprocal (fp16) -> rank-1 broadcast
                    r_sb = sbp.tile([128, 4, W], F16, tag="r_sb")
                    with nc.allow_low_precision(reason="softmax recip fp16"):
                        nc.vector.reciprocal(out=r_sb, in_=d_ps)
                    attn_sb = sbp.tile([128, 8, W], F16, tag="attn")
                    for j in range(8):
                        bc = pbc.tile([128, W], F32, tag="bc")
                        nc.tensor.matmul(bc, sel2[:, j % 2, :],
                                         r_sb[:, j // 2, :],
                                         start=True, stop=True)
                        nc.vector.tensor_tensor(
                            out=attn_sb[:, j, :], in0=pv_sb[:, j, :],
                            in1=bc, op=OP.mult)
                    nc.sync.dma_start(
                        out=scr[:, W * w:W * (w + 1)].rearrange(
                            "(c p) t -> p c t", p=128),
                        in_=attn_sb)

        attn_pass(0)
        attn_pass(1)

        # ---------------- final projection pass ----------------
        with tc.tile_pool(name="wf", bufs=1) as wp, \
             tc.tile_pool(name="sbf", bufs=3) as sbp, \
             tc.tile_pool(name="pproj", bufs=3, space="PSUM") as pproj, \
             tc.tile_pool(name="ptr", bufs=2, space="PSUM") as ptr:
            wos = wp.tile([128, EC, E], F16)
            wouts = wp.tile([128, EC, E], F16)
            nc.sync.dma_start(out=wos, in_=wo.rearrange("(c p) n -> p c n", p=128))
            nc.sync.dma_start(out=wouts,
                              in_=wout.rearrange("(c p) n -> p c n", p=128))
            for tb in range(TCORE // 128):
                t0 = tb * 128
                a1 = sbp.tile([128, EC, 128], F16, tag="a1")
                a2 = sbp.tile([128, EC, 128], F16, tag="a2")
                nc.sync.dma_start(
                    out=a1, in_=s1t[:, t0:t0 + 128].rearrange(
                        "(c p) t -> p c t", p=128))
                nc.sync.dma_start(
                    out=a2, in_=s2t[:, 128 + t0:128 + t0 + 128].rearrange(
                        "(c p) t -> p c t", p=128))
                aa = sbp.tile([128, EC, 128], F16, tag="aa")
                nc.vector.tensor_add(aa, a1, a2)
                # o = (a1+a2) @ (0.5*Wo); lhsT = aa chunks (feature-major)
                ps_o = pproj.tile([128, 2, 512], F32, tag="proj", name=f"o{tb}")
                for half in range(2):
                    for c in range(EC):
                        nc.tensor.matmul(
                            ps_o[:, half, :], aa[:, c, :],
                            wos[:, c, half * 512:(half + 1) * 512],
                            start=(c == 0), stop=(c == EC - 1))
                xcb = sbp.tile([128, E], F32, tag="xcb")
                nc.sync.dma_start(out=xcb, in_=xc[t0:t0 + 128, :])
                y = sbp.tile([128, E], F32, tag="y")
                nc.vector.tensor_tensor(
                    out=y, in0=ps_o.rearrange("p a b -> p (a b)"), in1=xcb,
                    op=OP.add)
                # LayerNorm 1
                stats = sbp.tile([128, 2, 6], F32, tag="stats")
                for g in range(2):
                    nc.vector.bn_stats(out=stats[:, g, :],
                                       in_=y[:, g * 512:(g + 1) * 512])
                mv = sbp.tile([128, 2], F32, tag="mv")
                nc.vector.bn_aggr(out=mv, in_=stats)
                rstd = sbp.tile([128, 1], F32, tag="rstd")
                nc.scalar.activation(out=rstd, in_=mv[:, 1:2], func=AF.Sqrt,
                                     bias=eps_t, scale=1.0)
                nc.vector.reciprocal(out=rstd, in_=rstd)
                mh = sbp.tile([128, E], F32, tag="mh")
                nc.vector.tensor_scalar(
                    out=mh, in0=y, scalar1=mv[:, 0:1], scalar2=rstd,
                    op0=OP.subtract, op1=OP.mult)
                if use_g1:
                    nc.vector.tensor_tensor(out=mh, in0=mh, in1=g1b, op=OP.mult)
                if use_b1:
                    nc.vector.tensor_tensor(out=mh, in0=mh, in1=b1b, op=OP.add)
                mh16 = sbp.tile([128, E], F16, tag="mh16")
                nc.scalar.copy(mh16, mh)
                # transpose mh -> mhT (PE transpose per 128-chunk)
                mhT = sbp.tile([128, EC, 128], F16, tag="mhT")
                for c in range(EC):
                    ps_t = ptr.tile([128, 128], F16, tag="tr")
                    nc.tensor.transpose(ps_t, mh16[:, c * 128:(c + 1) * 128],
                                        id128)
                    eng = nc.vector if c % 2 == 0 else nc.scalar
                    (eng.tensor_copy if eng is nc.vector else eng.copy)(
                        mhT[:, c, :], ps_t)
                ps_z = pproj.tile([128, 2, 512], F32, tag="proj", name=f"z{tb}")
                for half in range(2):
                    for c in range(EC):
                        nc.tensor.matmul(
                            ps_z[:, half, :], mhT[:, c, :],
                            wouts[:, c, half * 512:(half + 1) * 512],
                            start=(c == 0), stop=(c == EC - 1))
                z = sbp.tile([128, E], F32, tag="z")
                nc.vector.tensor_tensor(
                    out=z, in0=ps_z.rearrange("p a b -> p (a b)"), in1=mh,
                    op=OP.add)
                if use_bout:
                    nc.vector.tensor_tensor(out=z, in0=z, in1=boutb, op=OP.add)
                # LayerNorm 2 (+ relu fused into ACT when no affine)
                stats2 = sbp.tile([128, 2, 6], F32, tag="stats2")
                for g in range(2):
                    nc.vector.bn_stats(out=stats2[:, g, :],
                                       in_=z[:, g * 512:(g + 1) * 512])
                mv2 = sbp.tile([128, 2], F32, tag="mv2")
                nc.vector.bn_aggr(out=mv2, in_=stats2)
                rstd2 = sbp.tile([128, 1], F32, tag="rstd2")
                nc.scalar.activation(out=rstd2, in_=mv2[:, 1:2], func=AF.Sqrt,
                                     bias=eps_t, scale=1.0)
                nc.vector.reciprocal(out=rstd2, in_=rstd2)
                ob = sbp.tile([128, E], F32, tag="ob")
                if not (use_g2 or use_b2):
                    nmr = sbp.tile([128, 1], F32, tag="nmr")
                    nc.vector.tensor_scalar(
                        out=nmr, in0=mv2[:, 0:1], scalar1=rstd2, scalar2=-1.0,
                        op0=OP.mult, op1=OP.mult)
                    nc.scalar.activation(out=ob, in_=z, func=AF.Relu,
                                         bias=nmr, scale=rstd2)
                else:
                    nc.vector.tensor_scalar(
                        out=ob, in0=z, scalar1=mv2[:, 0:1], scalar2=rstd2,
                        op0=OP.subtract, op1=OP.mult)
                    if use_g2:
                        nc.vector.tensor_tensor(out=ob, in0=ob, in1=g2b,
                                                op=OP.mult)
                    if use_b2:
                        nc.vector.tensor_tensor(out=ob, in0=ob, in1=b2b,
                                                op=OP.add)
                    nc.vector.tensor_relu(out=ob, in_=ob)
                nc.sync.dma_start(out=out[t0:t0 + 128, :], in_=ob)
        cp.__exit__(None, None, None)

    nc.compile()
    return nc


def _get_program(flags):
    if flags not in _cache:
        _cache[flags] = _build(flags)
    return _cache[flags]


def kernel(x, W_q, W_k, W_v, W_o, W_out, b_out,
           ln1_g, ln1_b, ln2_g, ln2_b, _trace=False):
    x = np.asarray(x, dtype=np.float32)
    W_q = np.asarray(W_q, dtype=np.float32)
    W_k = np.asarray(W_k, dtype=np.float32)
    W_v = np.asarray(W_v, dtype=np.float32)
    W_o = np.asarray(W_o, dtype=np.float32)
    W_out = np.asarray(W_out, dtype=np.float32)
    b_out = np.asarray(b_out, dtype=np.float32)
    ln1_g = np.asarray(ln1_g, dtype=np.float32)
    ln1_b = np.asarray(ln1_b, dtype=np.float32)
    ln2_g = np.asarray(ln2_g, dtype=np.float32)
    ln2_b = np.asarray(ln2_b, dtype=np.float32)

    B, L, Ein = x.shape
    assert (B, L, Ein) == (4, 8192, E), (B, L, Ein)

    flags = (not np.all(ln1_g == 1.0), not np.all(ln1_b == 0.0),
             not np.all(ln2_g == 1.0), not np.all(ln2_b == 0.0),
             not np.all(b_out == 0.0))
    nc = _get_program(flags)

    dh_scale = np.float32(1.0 / np.sqrt(64.0))
    shared = {
        "wq0": (W_q[0] * dh_scale).astype(np.float16),
        "wq1": (W_q[1] * dh_scale).astype(np.float16),
        "wk0": W_k[0].astype(np.float16),
        "wk1": W_k[1].astype(np.float16),
        "wv0": W_v[0].astype(np.float16),
        "wv1": W_v[1].astype(np.float16),
        "wo": (W_o * np.float32(0.5)).astype(np.float16),
        "wout": W_out.astype(np.float16),
    }
    if flags[0]:
        shared["g1v"] = ln1_g
    if flags[1]:
        shared["b1v"] = ln1_b
    if flags[2]:
        shared["g2v"] = ln2_g
    if flags[3]:
        shared["b2v"] = ln2_b
    if flags[4]:
        shared["boutv"] = b_out

    xpad = np.zeros((B, L + 256, E), dtype=np.float32)
    xpad[:, 128:128 + L] = x
    in_maps = []
    for core in range(N_CORES):
        b, h = divmod(core, 2)
        r0 = h * TCORE
        ext = xpad[b, r0:r0 + TEXT]                      # [4352, 1024]
        m = dict(shared)
        m["xt"] = np.ascontiguousarray(ext.T).astype(np.float16)
        m["xc"] = np.ascontiguousarray(x[b, r0:r0 + TCORE])
        in_maps.append(m)

    res = run_bass_kernel_spmd(nc, in_maps, list(range(N_CORES)),
                               trace=_trace)
    out = np.empty((B, L, E), dtype=np.float32)
    for core in range(N_CORES):
        b, h = divmod(core, 2)
        out[b, h * TCORE:(h + 1) * TCORE] = res.results[core]["out"]
    if _trace:
        kernel.last_results = res
    return out
